# revision 28
# baseline (speedup 1.0000x reference)
"""Trainium2 Bass kernel for ContinuousIntegratedKoopmanOperator.

reference: odeint(dz/dt = z @ W) sampled at t = DT*[1..T], y0 = x at t[0].
Closed form (time-invariant linear ODE): out[:, j, :] = x @ expm(DT*j*W).

Strategy (final: ~55.1-56.2us traced / ~52us untraced, rel err 1.04e-3,
vs the 64.5us/3.6e-4 fp16 store-everything baseline):
  * device computes per-step DELTAS d_j = x @ (M^j - M^{j-1}), j=1..63,
    pre-scaled per-column (power-of-2) to ~unit std; stores fp8e3 (e3m4)
    -> 8.4MB/core stores (half the f16 baseline). Host decodes + cumsums
    from exact f32 x, so per-step quantization errors random-walk:
    ~1.3e-3 rel total (gate is 2e-2).
  * pipeline is rate-matched at FULL PE clock: V+S drains (PSUM f32 ->
    fp8e3 at 1 elem/lane/cycle, ~1.96 col/ns combined) are the consumer
    ceiling; fp16 self-loading 504-col matmuls run ~294ns (exposed
    ldweights) = 1.71 col/ns, so drains stay ahead and nobody blocks
    (a blocked engine pays 1.3-2.5us wake latency; the all-fp8-DoubleRow
    variant at 2.3+ col/ns convoyed to 74us that way, and PE stalls also
    reset the p-state ramp, making producer speed bimodal 1.2/2.4GHz).
  * x + md loads split across the sync and gpsimd DMA rings so the
    tile-0 column-chunk gates are satisfied before the PE gets there.
  * drains in 1024-col PAIRS (2 psum banks incl 8-col pads), greedy V/S
    split; small drain units let the PE hold a 4-6 unit lead inside the
    8-bank psum so every drain's s_mm wait is pre-satisfied (2048-col
    quads left no lead room: every drain blocked, paying ~1.5us wake
    that fed back through the psum-reuse gates -> 74-88us convoys).
    V/S warmup copies sized so the first gated drain's sem is already
    satisfied when checked. md chunk loads ride the SCALAR DGE ring:
    the gpsimd ring is clobbered by the NTFF profiler when tracing.
  * trailing dummy matmuls keep the PE busy through the store tail:
    HAM drops to k=4 (DMA crawls) ~3.6us after the PE goes idle; leading
    dummies are sized so the PE reaches unit 0's load gate AFTER its
    semaphore arrived (sleeping into the gate costs a ~1.3us wake), with
    the critical 516KB md chunk0 DMA triggered first in the load order.
  * 16 half-tile stores (512KB) so the DMA tail drains wide.
"""
import numpy as np
import ml_dtypes

DT = 0.01
B, D, T = 8192, 128, 64
NCORES = 8
BSH = B // NCORES          # 1024 rows per core
NTILES = BSH // 128        # 8 batch tiles per core
NJ = T - 1                 # j=1..63 on device; j=0 is x itself (host)
CT = NJ * D                # 8064 real output cols per row
UW = 504                   # matmul unit width (fits a 512-col psum bank)
NU_T = CT // UW            # 16 units per tile
NUNITS = NTILES * NU_T     # 128 units
SLOT = 512                 # psum bank width (f32 cols); unit u -> bank u%8
PADW = SLOT * NU_T         # 8192 staged cols per tile (incl 8-col pads)
NPAIR = NUNITS // 2        # 64 drain pairs (2 banks = 1024 cols each);
                           # small drain units let the PE hold a 4-6 unit
                           # lead within the 8-bank psum, keeping every
                           # drain's s_mm wait pre-satisfied (a blocked
                           # drain pays ~1.5us wake latency that feeds
                           # back through the psum-reuse gates)
MCW = 4 * UW               # 2016-col md load chunks (= 4 units)
DUMW = 13                  # leading PE warmup matmuls, sized to end just
                           # AFTER unit 0's load gate is satisfiable: a PE
                           # that sleeps into the gate pays ~1.3us wake
TRAILD = 8                 # trailing dummies: hold HAM k=8 through store tail
TSTD = 1.4                 # target per-column std of device outputs (e3m4)
SPLIT_EVERY = 5            # every Nth unit issued as 2 half-matmuls
                           # (+~130ns): trims the f16 producer (254ns/unit
                           # = 1.98 col/ns) to ~1.8, just under the paired
                           # V+S drain capacity (~1.83 col/ns), so drains
                           # run slightly behind a standing PE lead and
                           # their waits are pre-satisfied

# static drain-pair engine assignment (greedy by finish time, measured durs)
_DUR = {"V": 1217.0, "S": 983.0}
ENG_OF, IDX_OF = [], []
_fin = {"V": 0.0, "S": 0.0}
_cnt = {"V": 0, "S": 0}
for _q in range(NPAIR):
    _e = min(("S", "V"), key=lambda e: _fin[e] + _DUR[e])
    _fin[_e] += _DUR[_e]
    _cnt[_e] += 1
    ENG_OF.append(_e)
    IDX_OF.append(_cnt[_e])

_CACHE = {}


def _host_tables(W: np.ndarray):
    """float64 delta table -> (md16 [128, CT] f16, s2 f32 [NJ, D] scales)."""
    A = DT * W.astype(np.float64)
    M1 = np.eye(D)
    term = np.eye(D)
    for n in range(1, 30):
        term = term @ A / n
        M1 += term
    E = M1 - np.eye(D)
    Dp = np.empty((D, CT), dtype=np.float64)  # scaled deltas, j-major cols
    s2 = np.empty((NJ, D), dtype=np.float32)
    P = np.eye(D)                             # M^{j-1}
    for j in range(1, T):
        Dj = P @ E                            # M^{j-1} (M - I)
        P = P @ M1
        cn = np.linalg.norm(Dj, axis=0) / TSTD
        sc = np.exp2(np.round(np.log2(cn)))
        s2[j - 1] = sc.astype(np.float32)
        Dp[:, (j - 1) * D:j * D] = Dj / sc[None, :]
    return Dp.astype(np.float16), s2


def _build_nc():
    import concourse.bass as bass
    import concourse.mybir as mybir

    f32 = mybir.dt.float32
    f16 = mybir.dt.float16
    f8e3 = mybir.dt.float8e3

    nc = bass.Bass(trn_type="TRN2")
    xt_d = nc.dram_tensor("xt", (D, BSH), f16, kind="ExternalInput")
    md_d = nc.dram_tensor("md", (D, CT), f16, kind="ExternalInput")
    out_d = nc.dram_tensor("out8", (BSH, PADW), f8e3, kind="ExternalOutput")

    xt_s = nc.alloc_sbuf_tensor("xt_s", [D, BSH], f16)
    md_s = nc.alloc_sbuf_tensor("md_s", [D, CT], f16)
    stg = [nc.alloc_sbuf_tensor(f"stg{i}", [128, PADW], f8e3) for i in range(NTILES)]
    scr_v = nc.alloc_sbuf_tensor("scr_v", [128, 4224], f8e3)
    scr_s = nc.alloc_sbuf_tensor("scr_s", [128, 5824], f8e3)
    psum = nc.alloc_psum_tensor("acc", [128, 8 * SLOT], f32)

    s_ld = nc.alloc_semaphore("s_ld")    # sync-ring loads (xt)
    s_lg = nc.alloc_semaphore("s_lg")    # gpsimd-ring loads (md chunks)
    s_mm = nc.alloc_semaphore("s_mm")
    s_dv = nc.alloc_semaphore("s_dv")
    s_da = nc.alloc_semaphore("s_da")
    s_out = nc.alloc_semaphore("s_out")
    s_boot = nc.alloc_semaphore("s_boot")
    all_sems = [s_ld, s_lg, s_mm, s_dv, s_da, s_out, s_boot]
    nums = sorted(s.num for s in all_sems)
    assert nums == list(range(nums[0], nums[-1] + 1)), "sems not contiguous"
    sem_range = range(nums[0], nums[-1] + 1)
    nc.gpsimd.dma_reset(sem_range)

    def pair_wait(eng, p):
        eng.wait_ge(s_dv if ENG_OF[p] == "V" else s_da, IDX_OF[p])

    with nc.Block() as block:
        @block.sync
        def _(sync):
            sync.sem_clear(sem_range)
            sync.nop().then_inc(s_boot, 1)
            # md chunk0 is the critical load gating unit 0: its two halves
            # ride BOTH rings in parallel (sync here, scalar below), both
            # gated only at unit 0 -- no sequential mid-stream gate
            sync.dma_start(out=md_s[:, 0:MCW // 2], in_=md_d[:, 0:MCW // 2]).then_inc(s_ld, 16)
            sync.dma_start(out=xt_s[:, 0:128], in_=xt_d[:, 0:128]).then_inc(s_ld, 16)
            sync.dma_start(out=xt_s[:, 128:BSH], in_=xt_d[:, 128:BSH]).then_inc(s_ld, 16)
            for st in range(2 * NTILES):
                p_hi = 4 * st + 3
                cv = sum(1 for p in range(p_hi + 1) if ENG_OF[p] == "V")
                ca = (p_hi + 1) - cv
                if cv:
                    sync.wait_ge(s_dv, cv)
                if ca:
                    sync.wait_ge(s_da, ca)
                t, h = st // 2, st % 2
                sync.dma_start(
                    out=out_d[t * 128:(t + 1) * 128, h * 4096:(h + 1) * 4096],
                    in_=stg[t][:, h * 4096:(h + 1) * 4096],
                ).then_inc(s_out, 16)
            sync.wait_ge(s_out, 16 * 2 * NTILES)

        @block.scalar
        def _(scalar):
            # md chunks 1-3 ride the scalar DGE ring (the gpsimd ring is
            # clobbered by the NTFF profiler when tracing); the trigger
            # instrs + one warmup copy fill the time until the first
            # gated drain's sem arrives
            scalar.wait_ge(s_boot, 1)
            scalar.dma_start(out=md_s[:, MCW // 2:MCW],
                             in_=md_d[:, MCW // 2:MCW]).then_inc(s_lg, 16)
            for c in range(1, 4):
                scalar.dma_start(out=md_s[:, c * MCW:(c + 1) * MCW],
                                 in_=md_d[:, c * MCW:(c + 1) * MCW]).then_inc(s_lg, 16)
            scalar.copy(out=scr_s[:, 0:2912], in_=scr_s[:, 2912:5824])
            for p in range(NPAIR):
                if ENG_OF[p] != "S":
                    continue
                scalar.wait_ge(s_mm, 2 * p + 2)
                po = (2 * p % 8) * SLOT
                scalar.copy(out=stg[p // 8][:, (p % 8) * 1024:(p % 8 + 1) * 1024],
                            in_=psum[:, po:po + 1024]).then_inc(s_da, 1)

        @block.vector
        def _(vector):
            for _ in range(3):
                vector.tensor_copy(out=scr_v[:, 0:2112], in_=scr_v[:, 2112:4224])
            vector.wait_ge(s_boot, 1)
            for p in range(NPAIR):
                if ENG_OF[p] != "V":
                    continue
                vector.wait_ge(s_mm, 2 * p + 2)
                po = (2 * p % 8) * SLOT
                vector.tensor_copy(out=stg[p // 8][:, (p % 8) * 1024:(p % 8 + 1) * 1024],
                                   in_=psum[:, po:po + 1024]).then_inc(s_dv, 1)

        @block.tensor
        def _(tensor):
            # leading dummies: warm the PE p-state + HAM through the NEFF
            # preamble/load window; 512-wide so every psum col (incl pads)
            # is initialized before any drain reads it.
            for k in range(DUMW):
                tensor.matmul(psum[:, (k % 8) * SLOT:(k % 8) * SLOT + SLOT],
                              xt_s[:, 0:128], md_s[:, 0:SLOT],
                              start=True, stop=True)
            tensor.wait_ge(s_boot, 1)
            for u in range(NUNITS):
                t, b = u // NU_T, u % NU_T
                if u == 0:
                    tensor.wait_ge(s_ld, 32)   # xt tile0 + md chunk0 lo
                    tensor.wait_ge(s_lg, 16)   # md chunk0 hi (scalar ring)
                if b % 4 == 0 and b // 4 >= 1:
                    # md chunk gate for EVERY tile (pre-satisfied after
                    # tile 0, but enforces load/compute ordering)
                    tensor.wait_ge(s_lg, 16 * (b // 4 + 1))
                if u == NU_T:
                    tensor.wait_ge(s_ld, 48)   # xt rest loaded
                if u >= 8 and u % 2 == 0:
                    pair_wait(tensor, (u - 8) // 2)
                po = (u % 8) * SLOT
                xt_t = xt_s[:, t * 128:(t + 1) * 128]
                if u % SPLIT_EVERY == SPLIT_EVERY - 1:
                    h = UW // 2
                    tensor.matmul(psum[:, po:po + h], xt_t,
                                  md_s[:, b * UW:b * UW + h],
                                  start=True, stop=True)
                    mm = tensor.matmul(psum[:, po + h:po + UW], xt_t,
                                       md_s[:, b * UW + h:(b + 1) * UW],
                                       start=True, stop=True)
                else:
                    mm = tensor.matmul(psum[:, po:po + UW], xt_t,
                                       md_s[:, b * UW:(b + 1) * UW],
                                       start=True, stop=True)
                mm.then_inc(s_mm, 1)
            # trailing dummies: HAM drops DMA to k=4 ~3.6us after PE idles,
            # which crawls the store tail. Keep streaming garbage matmuls.
            for k in range(TRAILD):
                if k < 8 and k % 2 == 0:
                    pair_wait(tensor, 60 + k // 2)
                tensor.matmul(psum[:, (k % 8) * SLOT:(k % 8) * SLOT + SLOT],
                              xt_s[:, 0:128], md_s[:, 0:SLOT],
                              start=True, stop=True)

    return nc


def _prep_inputs(x: np.ndarray, md16):
    maps = []
    for c in range(NCORES):
        xc = x[c * BSH:(c + 1) * BSH]                       # (BSH, D) f32
        xt = np.ascontiguousarray(xc.T.astype(np.float16))
        maps.append({"xt": xt, "md": md16})
    return maps


def run_on_device(x: np.ndarray, tables, trace: bool = False):
    from concourse.bass_utils import run_bass_kernel_spmd

    md16, s2 = tables
    if "nc" not in _CACHE:
        _CACHE["nc"] = _build_nc()
    nc = _CACHE["nc"]

    in_maps = _prep_inputs(x, md16)
    res = run_bass_kernel_spmd(nc, in_maps, core_ids=list(range(NCORES)), trace=trace)
    out = np.empty((B, T, D), dtype=np.float32)
    for c in range(NCORES):
        xc = x[c * BSH:(c + 1) * BSH].astype(np.float32)
        raw = res.results[c]["out8"].astype(np.float32)     # (BSH, PADW)
        d = raw.reshape(BSH, NU_T, SLOT)[:, :, :UW].reshape(BSH, NJ, D)
        d *= s2[None, :, :]
        out[c * BSH:(c + 1) * BSH, 0] = xc
        out[c * BSH:(c + 1) * BSH, 1:] = xc[:, None, :] + np.cumsum(d, axis=1)
    return out, res


def kernel(x, W, T):
    x = np.asarray(x, dtype=np.float32)
    W = np.asarray(W, dtype=np.float32)
    assert int(T) == 64 and x.shape == (B, D) and W.shape == (D, D)
    tables = _host_tables(W)
    out, _ = run_on_device(x, tables, trace=False)
    return out
